# revision 10
# baseline (speedup 1.0000x reference)
"""Multi-head attention (B=2, S=2048, D=1024, H=16) on 8 Trainium2 cores.

Sharding: tensor-parallel over heads for QKV+attention (2 heads/core),
then an AllToAll reshards the attention output so each core computes the
output projection for its own 256-row slice of the sequence (both batches).
Host assembles the full output by concatenating the per-core slices.

Device pipeline per core (SPMD, identical program, shard-specific data):
  1. QK projection  : fp32r matmuls, outputs Q^T/K^T [128(2 heads), 2048]
  2. V projection   : bf16 matmuls (N=128), V stored [k, d] with a ones
                      column appended -> softmax denominator comes free
                      out of the attention*V matmul (M=65).
  3. Attention      : scores S^T = K^T.T @ Q^T (K=64 row-packed pairs via
                      tile_position), exp on ACT from 2-bank PSUM,
                      AV accumulation over 16 k-chunks, normalize with
                      reciprocal + partition_broadcast.
  4. AllToAll (bf16): O^T head-shards -> full-D s-slices.
  5. Out projection : bf16 matmuls + bias.

BASS_KSTAGE env (debug only): 1=QK proj, 2=+Vproj, 3=+attention,
4=+AllToAll, 5=full (default).
"""
import os
import sys

sys.path.insert(0, "/opt/trn_rl_repo")

import numpy as np
import ml_dtypes

import concourse.bass as bass
import concourse.tile as tile
from concourse import bacc, mybir
from concourse import bass_utils

B = 2
S = 2048
D = 1024
H = 16
DH = 64
N_CORES = 8
HEADS_PER_CORE = H // N_CORES          # 2
S_SLICE = S // N_CORES                 # 256
N_CH = D // 128                        # 8 contraction chunks
N_QT = S // 512                        # 4 q tiles
N_KC = S // 128                        # 16 k chunks

F32 = mybir.dt.float32
F32R = mybir.dt.float32r
BF16 = mybir.dt.bfloat16

_compiled = None
last_results = None


def _build():
    stage = int(os.environ.get("BASS_KSTAGE", "5"))
    sub = os.environ.get("BASS_KSUB", "")
    nc = bacc.Bacc(
        "TRN2",
        target_bir_lowering=False,
        debug=False,
        enable_asserts=True,
        num_devices=N_CORES,
    )

    xt = nc.dram_tensor("xt", [B, 128, N_CH, S], F32R, kind="ExternalInput").ap()
    xtb = nc.dram_tensor("xtb", [B, 128, N_CH, S], BF16, kind="ExternalInput").ap()
    wqt = nc.dram_tensor("wqt", [128, N_CH, 128], F32R, kind="ExternalInput").ap()
    wkt = nc.dram_tensor("wkt", [128, N_CH, 128], F32R, kind="ExternalInput").ap()
    wvt = nc.dram_tensor("wvt", [128, N_CH, 128], BF16, kind="ExternalInput").ap()
    wot = nc.dram_tensor("wot", [128, N_CH, D], BF16, kind="ExternalInput").ap()
    bb = nc.dram_tensor("bb", [128, D], F32, kind="ExternalInput").ap()
    oc = nc.dram_tensor("oc", [B, S_SLICE, D], F32, kind="ExternalOutput").ap()
    if stage < 5:
        dbg = nc.dram_tensor(
            "dbg", [B, N_CORES, 128, S_SLICE], BF16, kind="ExternalOutput"
        ).ap()
        dbgf = nc.dram_tensor(
            "dbgf", [5, 128, N_KC * 130], F32, kind="ExternalOutput"
        ).ap()

    EXP = mybir.ActivationFunctionType.Exp
    SCALE = DH ** -0.5

    with tile.TileContext(nc) as tc:
        with (
            tc.tile_pool(name="w", bufs=1) as wp,
            tc.tile_pool(name="qkt", bufs=1) as qktp,
            tc.tile_pool(name="vsb", bufs=1) as vsbp,
            tc.tile_pool(name="xtb", bufs=1) as xtbp,
            tc.tile_pool(name="xin", bufs=3) as xin,
            tc.tile_pool(name="pt", bufs=2) as ptp,
            tc.tile_pool(name="norm", bufs=2) as normp,
            tc.tile_pool(name="x2", bufs=1) as x2p,
            tc.tile_pool(name="outsb", bufs=2) as outp,
            tc.tile_pool(name="dram", bufs=1, space="DRAM") as dram,
        ):
            # ---- weights ----
            wqt_sb = wp.tile([128, N_CH * 128], F32R)
            nc.sync.dma_start(wqt_sb[:], wqt[:].rearrange("p c e -> p (c e)"))
            wkt_sb = wp.tile([128, N_CH * 128], F32R)
            nc.sync.dma_start(wkt_sb[:], wkt[:].rearrange("p c e -> p (c e)"))
            wvt_sb = wp.tile([128, N_CH * 128], BF16)
            nc.sync.dma_start(wvt_sb[:], wvt[:].rearrange("p c e -> p (c e)"))

            # ---- phase B: QK projection ----
            Qt, Kt = [], []
            with tc.tile_pool(name="qkps", bufs=2, space="PSUM") as qkps:
                for b in range(B):
                    qt_sb = qktp.tile([128, S], F32R, tag=f"qt{b}", name=f"qt{b}")
                    kt_sb = qktp.tile([128, S], F32R, tag=f"kt{b}", name=f"kt{b}")
                    Qt.append(qt_sb)
                    Kt.append(kt_sb)
                    for t in range(N_QT):
                        q_ps = qkps.tile([128, 512], F32, tag="q", name="q_ps")
                        k_ps = qkps.tile([128, 512], F32, tag="k", name="k_ps")
                        for ch in range(N_CH):
                            xt_t = xin.tile([128, 512], F32R, name="xt_t")
                            nc.sync.dma_start(
                                xt_t[:], xt[b, :, ch, t * 512:(t + 1) * 512]
                            )
                            nc.tensor.matmul(
                                q_ps[:],
                                lhsT=wqt_sb[:, ch * 128:(ch + 1) * 128],
                                rhs=xt_t[:],
                                start=(ch == 0),
                                stop=(ch == N_CH - 1),
                            )
                            nc.tensor.matmul(
                                k_ps[:],
                                lhsT=wkt_sb[:, ch * 128:(ch + 1) * 128],
                                rhs=xt_t[:],
                                start=(ch == 0),
                                stop=(ch == N_CH - 1),
                            )
                        nc.vector.tensor_copy(
                            qt_sb[:, t * 512:(t + 1) * 512], q_ps[:]
                        )
                        nc.vector.tensor_copy(
                            kt_sb[:, t * 512:(t + 1) * 512], k_ps[:]
                        )
            if stage == 1:
                for i, t_sb in enumerate([Qt[0], Kt[0], Qt[1], Kt[1]]):
                    nc.sync.dma_start(dbgf[i, :, 0:S], t_sb[:].bitcast(F32))

            # ---- phases C/D/E share the remaining 8 PSUM banks ----
            with (
                tc.tile_pool(name="vps", bufs=1, space="PSUM") as vps,
                tc.tile_pool(name="sps", bufs=1, space="PSUM") as sps,
                tc.tile_pool(name="avps", bufs=1, space="PSUM") as avps,
                tc.tile_pool(name="ops", bufs=1, space="PSUM") as ops,
            ):
                # ---- phase C: V projection (bf16) ----
                Vs = []
                if stage >= 2:
                    for b in range(B):
                        xtb_sb = xtbp.tile([128, N_CH * S], BF16, tag="xtb",
                                           name="xtb_sb")
                        nc.sync.dma_start(
                            xtb_sb[:], xtb[b].rearrange("p c s -> p (c s)")
                        )
                        v_sb = vsbp.tile([128, N_KC * 130], F32R, tag=f"v{b}",
                                         name=f"v{b}")
                        Vs.append(v_sb)
                        ones_ap = v_sb[:].rearrange("p (c o) -> p c o", o=65)[
                            :, :, 64:65
                        ]
                        nc.gpsimd.memset(ones_ap.bitcast(F32), 1.0)
                        for st in range(N_KC):
                            v_ps = vps.tile([128, 128], F32, tag="v", name="v_ps")
                            for ch in range(N_CH):
                                nc.tensor.matmul(
                                    v_ps[:],
                                    lhsT=xtb_sb[:, ch * S + st * 128:
                                                ch * S + (st + 1) * 128],
                                    rhs=wvt_sb[:, ch * 128:(ch + 1) * 128],
                                    start=(ch == 0),
                                    stop=(ch == N_CH - 1),
                                )
                            dst = v_sb[:].rearrange("p (c o) -> p c o", o=65)[
                                :, 2 * st:2 * st + 2, 0:64
                            ]
                            nc.vector.tensor_copy(
                                dst, v_ps[:].rearrange("p (h e) -> p h e", e=64)
                            )
                if stage == 2:
                    for b in range(B):
                        nc.sync.dma_start(
                            dbgf[b, :, 0:N_KC * 130], Vs[b][:].bitcast(F32)
                        )

                # ---- phase D: attention + A2A per batch ----
                if stage >= 3:
                    a2a_in = [
                        dram.tile([N_CORES, 128, S_SLICE], BF16,
                                  tag=f"a2ai{b}", name=f"a2ai{b}")
                        for b in range(B)
                    ]
                    a2a_out = [
                        dram.tile([N_CORES, 128, S_SLICE], BF16,
                                  tag=f"a2ao{b}", name=f"a2ao{b}")
                        for b in range(B)
                    ]
                    for b in range(B):
                        for t in range(N_QT):
                            av_A = avps.tile([65, 512], F32, tag="avA", name="av_A")
                            av_B = avps.tile([65, 512], F32, tag="avB", name="av_B")
                            qs = slice(t * 512, (t + 1) * 512)
                            for cc in range(N_KC // 2):
                                s_A = sps.tile([128, 1024], F32, tag="sA", name="s_A")
                                s_B = sps.tile([128, 1024], F32, tag="sB", name="s_B")
                                for j in range(2):
                                    c = 2 * cc + j
                                    ks = slice(c * 128, (c + 1) * 128)
                                    nc.tensor.matmul(
                                        s_A[:, j * 512:(j + 1) * 512],
                                        lhsT=Kt[b][0:64, ks],
                                        rhs=Qt[b][0:64, qs],
                                        start=True, stop=True,
                                        tile_position=(0, 0),
                                    )
                                    nc.tensor.matmul(
                                        s_B[:, j * 512:(j + 1) * 512],
                                        lhsT=Kt[b][64:128, ks],
                                        rhs=Qt[b][64:128, qs],
                                        start=True, stop=True,
                                        tile_position=(64, 0),
                                    )
                                p_A = ptp.tile([128, 1024], F32R, tag="pA",
                                               name="p_A")
                                nc.scalar.activation(p_A[:], s_A[:], EXP,
                                                     scale=SCALE)
                                p_B = ptp.tile([128, 1024], F32R, tag="pB",
                                               name="p_B")
                                nc.scalar.activation(p_B[:], s_B[:], EXP,
                                                     scale=SCALE)
                                for j in range(2 if sub not in ("a",) else 0):
                                    c = 2 * cc + j
                                    nc.tensor.matmul(
                                        av_A[:],
                                        lhsT=Vs[b][:, c * 130:c * 130 + 65],
                                        rhs=p_A[:, j * 512:(j + 1) * 512],
                                        start=(c == 0), stop=(c == N_KC - 1),
                                        skip_group_check=True,
                                    )
                                    nc.tensor.matmul(
                                        av_B[:],
                                        lhsT=Vs[b][:, c * 130 + 65:c * 130 + 130],
                                        rhs=p_B[:, j * 512:(j + 1) * 512],
                                        start=(c == 0), stop=(c == N_KC - 1),
                                        skip_group_check=True,
                                    )
                            for h, av in (() if sub in ("a", "b") else ((0, av_A), (1, av_B))):
                                recip = normp.tile([1, 512], F32, tag="rc",
                                                   name="recip")
                                nc.vector.reciprocal(recip[:], av[64:65, :])
                                bcast = normp.tile([64, 512], F32, tag="bc",
                                                   name="bcast")
                                nc.gpsimd.partition_broadcast(bcast[:], recip[:])
                                o_sb = normp.tile([64, 512], BF16, tag="ob",
                                                  name="o_sb")
                                nc.vector.tensor_mul(o_sb[:], av[0:64, :], bcast[:])
                                if sub == "c":
                                    nc.sync.dma_start(
                                        dbg[b, 2 * t, h * 64:(h + 1) * 64, :],
                                        o_sb[:, 0:S_SLICE],
                                    )
                                else:
                                    for j in range(2):
                                        nc.sync.dma_start(
                                            a2a_in[b][2 * t + j,
                                                      h * 64:(h + 1) * 64, :],
                                            o_sb[:, j * S_SLICE:(j + 1) * S_SLICE],
                                        )
                        if stage >= 4:
                            nc.gpsimd.collective_compute(
                                "AllToAll",
                                mybir.AluOpType.bypass,
                                replica_groups=[list(range(N_CORES))],
                                ins=[a2a_in[b][:]],
                                outs=[a2a_out[b][:]],
                            )
                if stage == 3 and sub == "":
                    for b in range(B):
                        nc.sync.dma_start(dbg[b], a2a_in[b][:])
                if stage == 4:
                    for b in range(B):
                        nc.sync.dma_start(dbg[b], a2a_out[b][:])

                # ---- phase E: output projection (bf16) ----
                if stage >= 5:
                    wot_sb = wp.tile([128, N_CH * D], BF16)
                    nc.sync.dma_start(wot_sb[:],
                                      wot[:].rearrange("p c e -> p (c e)"))
                    bb_sb = wp.tile([128, D], F32)
                    nc.sync.dma_start(bb_sb[:], bb[:])
                    for b in range(B):
                        x2 = []
                        for i in range(N_CH):
                            x2_sb = x2p.tile([128, S_SLICE], BF16,
                                             tag=f"x2_{b}_{i}", name=f"x2_{b}_{i}")
                            nc.sync.dma_start(x2_sb[:], a2a_out[b][i])
                            x2.append(x2_sb)
                        for st in range(S_SLICE // 128):
                            for et in range(D // 512):
                                o_ps = ops.tile([128, 512], F32, tag="o",
                                                name="o_ps")
                                for ch in range(N_CH):
                                    nc.tensor.matmul(
                                        o_ps[:],
                                        lhsT=x2[ch][:, st * 128:(st + 1) * 128],
                                        rhs=wot_sb[:, ch * D + et * 512:
                                                   ch * D + (et + 1) * 512],
                                        start=(ch == 0),
                                        stop=(ch == N_CH - 1),
                                    )
                                out_sb = outp.tile([128, 512], F32, tag="osb",
                                                   name="out_sb")
                                nc.vector.tensor_add(
                                    out_sb[:], o_ps[:],
                                    bb_sb[:, et * 512:(et + 1) * 512]
                                )
                                nc.sync.dma_start(
                                    oc[b, st * 128:(st + 1) * 128,
                                       et * 512:(et + 1) * 512],
                                    out_sb[:],
                                )

    nc.compile()
    return nc


def _prep_chunked(a_t):
    """[Din, E] (already transposed) -> [128, Din//128, E] SBUF-chunk layout."""
    din, e = a_t.shape
    return np.ascontiguousarray(
        a_t.reshape(din // 128, 128, e).transpose(1, 0, 2)
    )


def kernel(x, w_qkv, w_out, b_out):
    global _compiled, last_results
    if _compiled is None:
        _compiled = _build()
    nc = _compiled

    x = np.asarray(x, dtype=np.float32)
    w_qkv = np.asarray(w_qkv, dtype=np.float32)
    w_out = np.asarray(w_out, dtype=np.float32)
    b_out = np.asarray(b_out, dtype=np.float32)

    # x^T in chunk layout: [B, 128, N_CH, S]
    xt_full = x.transpose(0, 2, 1)  # [B, D, S]
    xt_prep = np.ascontiguousarray(
        xt_full.reshape(B, N_CH, 128, S).transpose(0, 2, 1, 3)
    )
    xtb_prep = xt_prep.astype(ml_dtypes.bfloat16)

    wot_prep = _prep_chunked(np.ascontiguousarray(w_out.T)).astype(ml_dtypes.bfloat16)
    bb_np = np.ascontiguousarray(np.broadcast_to(b_out, (128, D)))

    in_maps = []
    for c in range(N_CORES):
        hA, hB = HEADS_PER_CORE * c, HEADS_PER_CORE * c + 1
        rows = np.r_[hA * DH:(hA + 1) * DH, hB * DH:(hB + 1) * DH]
        wq = w_qkv[rows, :]               # [128, D]
        wk = w_qkv[D + rows, :]
        wv = w_qkv[2 * D + rows, :]
        in_maps.append({
            "xt": xt_prep,
            "xtb": xtb_prep,
            "wqt": _prep_chunked(np.ascontiguousarray(wq.T)),
            "wkt": _prep_chunked(np.ascontiguousarray(wk.T)),
            "wvt": _prep_chunked(np.ascontiguousarray(wv.T)).astype(ml_dtypes.bfloat16),
            "wot": wot_prep,
            "bb": bb_np,
        })

    last_results = bass_utils.run_bass_kernel_spmd(
        nc, in_maps, core_ids=list(range(N_CORES))
    )
    out = np.concatenate(
        [last_results.results[c]["oc"] for c in range(N_CORES)], axis=1
    )
    return out


# revision 13
# speedup vs baseline: 1.1614x; 1.1614x over previous
"""Multi-head attention (B=2, S=2048, D=1024, H=16) on 8 Trainium2 cores.

Sharding: tensor-parallel over heads for QKV+attention (2 heads/core),
then an AllToAll reshards the attention output so each core computes the
output projection for its own 256-row slice of the sequence (both batches).
Host assembles the full output by concatenating the per-core slices.

Device pipeline per core (SPMD, identical program, shard-specific data):
  1. QK projection  : fp32r matmuls, outputs Q^T/K^T [128(2 heads), 2048]
  2. V projection   : bf16 matmuls (N=128), V stored [k, d] with a ones
                      column appended -> softmax denominator comes free
                      out of the attention*V matmul (M=65).
  3. Attention      : scores S^T = K^T.T @ Q^T (K=64 row-packed pairs via
                      tile_position), exp on ACT from 2-bank PSUM,
                      AV accumulation over 16 k-chunks, normalize with
                      reciprocal + partition_broadcast.
  4. AllToAll (bf16): O^T head-shards -> full-D s-slices.
  5. Out projection : bf16 matmuls + bias.

BASS_KSTAGE env (debug only): 1=QK proj, 2=+Vproj, 3=+attention,
4=+AllToAll, 5=full (default).
"""
import os
import sys

sys.path.insert(0, "/opt/trn_rl_repo")

import numpy as np
import ml_dtypes

import concourse.bass as bass
import concourse.tile as tile
from concourse import bacc, mybir
from concourse import bass_utils

B = 2
S = 2048
D = 1024
H = 16
DH = 64
N_CORES = 8
HEADS_PER_CORE = H // N_CORES          # 2
S_SLICE = S // N_CORES                 # 256
N_CH = D // 128                        # 8 contraction chunks
N_QT = S // 512                        # 4 q tiles
N_KC = S // 128                        # 16 k chunks

F32 = mybir.dt.float32
F32R = mybir.dt.float32r
BF16 = mybir.dt.bfloat16

_compiled = None
last_results = None


def _build():
    stage = int(os.environ.get("BASS_KSTAGE", "5"))
    sub = os.environ.get("BASS_KSUB", "")
    nc = bacc.Bacc(
        "TRN2",
        target_bir_lowering=False,
        debug=False,
        enable_asserts=True,
        num_devices=N_CORES,
    )

    xt = nc.dram_tensor("xt", [B, 128, N_CH, S], F32R, kind="ExternalInput").ap()
    xtb = nc.dram_tensor("xtb", [B, 128, N_CH, S], BF16, kind="ExternalInput").ap()
    wqt = nc.dram_tensor("wqt", [128, N_CH, 128], F32R, kind="ExternalInput").ap()
    wkt = nc.dram_tensor("wkt", [128, N_CH, 128], F32R, kind="ExternalInput").ap()
    wvt = nc.dram_tensor("wvt", [128, N_CH, 128], BF16, kind="ExternalInput").ap()
    wot = nc.dram_tensor("wot", [128, N_CH, D], BF16, kind="ExternalInput").ap()
    bb = nc.dram_tensor("bb", [128, D], F32, kind="ExternalInput").ap()
    oc = nc.dram_tensor("oc", [B, S_SLICE, D], F32, kind="ExternalOutput").ap()
    if stage < 5:
        dbg = nc.dram_tensor(
            "dbg", [B, N_CORES, 128, S_SLICE], BF16, kind="ExternalOutput"
        ).ap()
        dbgf = nc.dram_tensor(
            "dbgf", [5, 128, N_KC * 130], F32, kind="ExternalOutput"
        ).ap()

    EXP = mybir.ActivationFunctionType.Exp
    SCALE = DH ** -0.5

    with tile.TileContext(nc) as tc:
        with (
            tc.tile_pool(name="w", bufs=1) as wp,
            tc.tile_pool(name="qkt", bufs=1) as qktp,
            tc.tile_pool(name="vsb", bufs=1) as vsbp,
            tc.tile_pool(name="xtb", bufs=1) as xtbp,
            tc.tile_pool(name="xin", bufs=3) as xin,
            tc.tile_pool(name="pt", bufs=2) as ptp,
            tc.tile_pool(name="norm", bufs=2) as normp,
            tc.tile_pool(name="x2", bufs=1) as x2p,
            tc.tile_pool(name="outsb", bufs=2) as outp,
            tc.tile_pool(name="dram", bufs=1, space="DRAM") as dram,
        ):
            # ---- weights ----
            wqt_sb = wp.tile([128, N_CH * 128], F32R)
            nc.sync.dma_start(wqt_sb[:], wqt[:].rearrange("p c e -> p (c e)"))
            wkt_sb = wp.tile([128, N_CH * 128], F32R)
            nc.sync.dma_start(wkt_sb[:], wkt[:].rearrange("p c e -> p (c e)"))
            wvt_sb = wp.tile([128, N_CH * 128], BF16)
            nc.sync.dma_start(wvt_sb[:], wvt[:].rearrange("p c e -> p (c e)"))

            # ---- phase B: QK projection ----
            Qt, Kt = [], []
            with tc.tile_pool(name="qkps", bufs=2, space="PSUM") as qkps:
                for b in range(B):
                    qt_sb = qktp.tile([128, S], BF16, tag=f"qt{b}", name=f"qt{b}")
                    kt_sb = qktp.tile([128, S], BF16, tag=f"kt{b}", name=f"kt{b}")
                    Qt.append(qt_sb)
                    Kt.append(kt_sb)
                    for t in range(N_QT):
                        q_ps = qkps.tile([128, 512], F32, tag="q", name="q_ps")
                        k_ps = qkps.tile([128, 512], F32, tag="k", name="k_ps")
                        for ch in range(N_CH):
                            xt_t = xin.tile([128, 512], F32R, name="xt_t")
                            nc.sync.dma_start(
                                xt_t[:], xt[b, :, ch, t * 512:(t + 1) * 512]
                            )
                            nc.tensor.matmul(
                                q_ps[:],
                                lhsT=wqt_sb[:, ch * 128:(ch + 1) * 128],
                                rhs=xt_t[:],
                                start=(ch == 0),
                                stop=(ch == N_CH - 1),
                            )
                            nc.tensor.matmul(
                                k_ps[:],
                                lhsT=wkt_sb[:, ch * 128:(ch + 1) * 128],
                                rhs=xt_t[:],
                                start=(ch == 0),
                                stop=(ch == N_CH - 1),
                            )
                        nc.vector.tensor_copy(
                            qt_sb[:, t * 512:(t + 1) * 512], q_ps[:]
                        )
                        nc.vector.tensor_copy(
                            kt_sb[:, t * 512:(t + 1) * 512], k_ps[:]
                        )
            if stage == 1:
                for i, t_sb in enumerate([Qt[0], Kt[0], Qt[1], Kt[1]]):
                    nc.sync.dma_start(dbgf[i, :, 0:S], t_sb[:].bitcast(F32))

            # ---- phases C/D/E share the remaining 8 PSUM banks ----
            with (
                tc.tile_pool(name="vps", bufs=1, space="PSUM") as vps,
                tc.tile_pool(name="sps", bufs=2, space="PSUM") as sps,
                tc.tile_pool(name="avps", bufs=2, space="PSUM") as avps,
                tc.tile_pool(name="ops", bufs=1, space="PSUM") as ops,
            ):
                # ---- phase C: V projection (bf16) ----
                Vs = []
                if stage >= 2:
                    for b in range(B):
                        xtb_sb = xtbp.tile([128, N_CH * S], BF16, tag="xtb",
                                           name="xtb_sb")
                        nc.sync.dma_start(
                            xtb_sb[:], xtb[b].rearrange("p c s -> p (c s)")
                        )
                        v_sb = vsbp.tile([128, N_KC * 130], BF16, tag=f"v{b}",
                                         name=f"v{b}")
                        Vs.append(v_sb)
                        ones_ap = v_sb[:].rearrange("p (c o) -> p c o", o=65)[
                            :, :, 64:65
                        ]
                        nc.gpsimd.memset(ones_ap, 1.0)
                        for st in range(N_KC):
                            v_ps = vps.tile([128, 128], F32, tag="v", name="v_ps")
                            for ch in range(N_CH):
                                nc.tensor.matmul(
                                    v_ps[:],
                                    lhsT=xtb_sb[:, ch * S + st * 128:
                                                ch * S + (st + 1) * 128],
                                    rhs=wvt_sb[:, ch * 128:(ch + 1) * 128],
                                    start=(ch == 0),
                                    stop=(ch == N_CH - 1),
                                )
                            dst = v_sb[:].rearrange("p (c o) -> p c o", o=65)[
                                :, 2 * st:2 * st + 2, 0:64
                            ]
                            nc.vector.tensor_copy(
                                dst, v_ps[:].rearrange("p (h e) -> p h e", e=64)
                            )
                if stage == 2:
                    for b in range(B):
                        nc.sync.dma_start(
                            dbgf[b, :, 0:N_KC * 130], Vs[b][:].bitcast(F32)
                        )

                # ---- phase D: attention + A2A per batch ----
                if stage >= 3:
                    a2a_in = [
                        dram.tile([N_CORES, 128, S_SLICE], BF16,
                                  tag=f"a2ai{b}", name=f"a2ai{b}")
                        for b in range(B)
                    ]
                    a2a_out = [
                        dram.tile([N_CORES, 128, S_SLICE], BF16,
                                  tag=f"a2ao{b}", name=f"a2ao{b}")
                        for b in range(B)
                    ]
                    for b in range(B):
                        for h in range(2):
                            hp = slice(h * 64, (h + 1) * 64)
                            voff = h * 65
                            for t in range(N_QT):
                                av = avps.tile([65, 512], F32, tag="av",
                                               name="av")
                                qs = slice(t * 512, (t + 1) * 512)
                                for cc in range(N_KC // 2):
                                    s_ps = sps.tile([128, 1024], F32, tag="s",
                                                    name="s_ps")
                                    for j in range(2):
                                        c = 2 * cc + j
                                        ks = slice(c * 128, (c + 1) * 128)
                                        nc.tensor.matmul(
                                            s_ps[:, j * 512:(j + 1) * 512],
                                            lhsT=Kt[b][hp, ks],
                                            rhs=Qt[b][hp, qs],
                                            start=True, stop=True,
                                        )
                                    p_sb = ptp.tile([128, 1024], BF16, tag="p",
                                                    name="p_sb")
                                    nc.scalar.activation(p_sb[:], s_ps[:], EXP,
                                                         scale=SCALE)
                                    for j in range(2 if sub not in ("a",) else 0):
                                        c = 2 * cc + j
                                        nc.tensor.matmul(
                                            av[:],
                                            lhsT=Vs[b][:, c * 130 + voff:
                                                       c * 130 + voff + 65],
                                            rhs=p_sb[:, j * 512:(j + 1) * 512],
                                            start=(c == 0), stop=(c == N_KC - 1),
                                            skip_group_check=True,
                                        )
                                if sub in ("a", "b"):
                                    continue
                                recip = normp.tile([1, 512], F32, tag="rc",
                                                   name="recip")
                                nc.vector.reciprocal(
                                    recip[:], av[64:65, :])
                                bcast = normp.tile([64, 512], F32, tag="bc",
                                                   name="bcast")
                                nc.gpsimd.partition_broadcast(bcast[:], recip[:])
                                o_sb = normp.tile([64, 512], BF16, tag="ob",
                                                  name="o_sb")
                                nc.vector.tensor_mul(o_sb[:], av[0:64, :],
                                                     bcast[:])
                                for j in range(2):
                                    nc.sync.dma_start(
                                        a2a_in[b][2 * t + j, hp, :],
                                        o_sb[:, j * S_SLICE:(j + 1) * S_SLICE],
                                    )
                        if stage >= 4:
                            nc.gpsimd.collective_compute(
                                "AllToAll",
                                mybir.AluOpType.bypass,
                                replica_groups=[list(range(N_CORES))],
                                ins=[a2a_in[b][:]],
                                outs=[a2a_out[b][:]],
                            )
                if stage == 3 and sub == "":
                    for b in range(B):
                        nc.sync.dma_start(dbg[b], a2a_in[b][:])
                if stage == 4:
                    for b in range(B):
                        nc.sync.dma_start(dbg[b], a2a_out[b][:])

                # ---- phase E: output projection (bf16) ----
                if stage >= 5:
                    wot_sb = wp.tile([128, N_CH * D], BF16)
                    nc.sync.dma_start(wot_sb[:],
                                      wot[:].rearrange("p c e -> p (c e)"))
                    bb_sb = wp.tile([128, D], F32)
                    nc.sync.dma_start(bb_sb[:], bb[:])
                    for b in range(B):
                        x2 = []
                        for i in range(N_CH):
                            x2_sb = x2p.tile([128, S_SLICE], BF16,
                                             tag=f"x2_{b}_{i}", name=f"x2_{b}_{i}")
                            nc.sync.dma_start(x2_sb[:], a2a_out[b][i])
                            x2.append(x2_sb)
                        for st in range(S_SLICE // 128):
                            for et in range(D // 512):
                                o_ps = ops.tile([128, 512], F32, tag="o",
                                                name="o_ps")
                                for ch in range(N_CH):
                                    nc.tensor.matmul(
                                        o_ps[:],
                                        lhsT=x2[ch][:, st * 128:(st + 1) * 128],
                                        rhs=wot_sb[:, ch * D + et * 512:
                                                   ch * D + (et + 1) * 512],
                                        start=(ch == 0),
                                        stop=(ch == N_CH - 1),
                                    )
                                out_sb = outp.tile([128, 512], F32, tag="osb",
                                                   name="out_sb")
                                nc.vector.tensor_add(
                                    out_sb[:], o_ps[:],
                                    bb_sb[:, et * 512:(et + 1) * 512]
                                )
                                nc.sync.dma_start(
                                    oc[b, st * 128:(st + 1) * 128,
                                       et * 512:(et + 1) * 512],
                                    out_sb[:],
                                )

    nc.compile()
    return nc


def _prep_chunked(a_t):
    """[Din, E] (already transposed) -> [128, Din//128, E] SBUF-chunk layout."""
    din, e = a_t.shape
    return np.ascontiguousarray(
        a_t.reshape(din // 128, 128, e).transpose(1, 0, 2)
    )


def kernel(x, w_qkv, w_out, b_out):
    global _compiled, last_results
    if _compiled is None:
        _compiled = _build()
    nc = _compiled

    x = np.asarray(x, dtype=np.float32)
    w_qkv = np.asarray(w_qkv, dtype=np.float32)
    w_out = np.asarray(w_out, dtype=np.float32)
    b_out = np.asarray(b_out, dtype=np.float32)

    # x^T in chunk layout: [B, 128, N_CH, S]
    xt_full = x.transpose(0, 2, 1)  # [B, D, S]
    xt_prep = np.ascontiguousarray(
        xt_full.reshape(B, N_CH, 128, S).transpose(0, 2, 1, 3)
    )
    xtb_prep = xt_prep.astype(ml_dtypes.bfloat16)

    wot_prep = _prep_chunked(np.ascontiguousarray(w_out.T)).astype(ml_dtypes.bfloat16)
    bb_np = np.ascontiguousarray(np.broadcast_to(b_out, (128, D)))

    in_maps = []
    for c in range(N_CORES):
        hA, hB = HEADS_PER_CORE * c, HEADS_PER_CORE * c + 1
        rows = np.r_[hA * DH:(hA + 1) * DH, hB * DH:(hB + 1) * DH]
        wq = w_qkv[rows, :]               # [128, D]
        wk = w_qkv[D + rows, :]
        wv = w_qkv[2 * D + rows, :]
        in_maps.append({
            "xt": xt_prep,
            "xtb": xtb_prep,
            "wqt": _prep_chunked(np.ascontiguousarray(wq.T)),
            "wkt": _prep_chunked(np.ascontiguousarray(wk.T)),
            "wvt": _prep_chunked(np.ascontiguousarray(wv.T)).astype(ml_dtypes.bfloat16),
            "wot": wot_prep,
            "bb": bb_np,
        })

    last_results = bass_utils.run_bass_kernel_spmd(
        nc, in_maps, core_ids=list(range(N_CORES))
    )
    out = np.concatenate(
        [last_results.results[c]["oc"] for c in range(N_CORES)], axis=1
    )
    return out


# revision 19
# speedup vs baseline: 1.5099x; 1.3001x over previous
"""Multi-head attention (B=2, S=2048, D=1024, H=16) on 8 Trainium2 cores.

Sharding: tensor-parallel over heads for QKV+attention (2 heads/core),
then an AllToAll reshards the attention output so each core computes the
output projection for its own 256-row slice of the sequence (both batches).
Host assembles the full output by concatenating the per-core slices.

Device pipeline per core (SPMD, identical program, shard-specific data):
  - starting-gun AllGather to absorb inter-core launch skew
  - PE warmup matmuls while x^T streams in (HAM clock ramp)
  - QKV projection (bf16): batch 0 in a ch-outer fast-start variant,
    batch 1 interleaved into batch-0 attention emission (PE executes its
    instruction stream in order)
  - attention: S^T = K^T.T @ Q^T, exp on ACT (PSUM [128,1024]), A*V with a
    ones column on V producing the softmax denominator (M=65), normalize
    via reciprocal on a [64,8] reshape + DMA partition-broadcast
  - AllToAll (bf16) per batch, out-projection overlapped.

PSUM budget (8 banks): tag s [128,1024]x2 = 4, tag av [65,512]x3 = 3,
tag o [128,512]x1 = 1. QK proj borrows s slots, V/warmup/outproj borrow o.
"""
import os
import sys

sys.path.insert(0, "/opt/trn_rl_repo")

import numpy as np
import ml_dtypes

import concourse.bass as bass
import concourse.tile as tile
from concourse import bacc, mybir
from concourse import bass_utils

B = 2
S = 2048
D = 1024
H = 16
DH = 64
N_CORES = 8
HEADS_PER_CORE = H // N_CORES          # 2
S_SLICE = S // N_CORES                 # 256
N_CH = D // 128                        # 8 contraction chunks
N_QT = S // 512                        # 4 q tiles
N_KC = S // 128                        # 16 k chunks

F32 = mybir.dt.float32
BF16 = mybir.dt.bfloat16

_compiled = None
last_results = None


def _build():
    nc = bacc.Bacc(
        "TRN2",
        target_bir_lowering=False,
        debug=False,
        enable_asserts=True,
        num_devices=N_CORES,
    )

    xtb = nc.dram_tensor("xtb", [B, 128, N_CH, S], BF16, kind="ExternalInput").ap()
    wqt = nc.dram_tensor("wqt", [128, N_CH, 128], BF16, kind="ExternalInput").ap()
    wkt = nc.dram_tensor("wkt", [128, N_CH, 128], BF16, kind="ExternalInput").ap()
    wvt = nc.dram_tensor("wvt", [128, N_CH, 128], BF16, kind="ExternalInput").ap()
    wot = nc.dram_tensor("wot", [128, N_CH, D], BF16, kind="ExternalInput").ap()
    bb = nc.dram_tensor("bb", [128, D], F32, kind="ExternalInput").ap()
    oc = nc.dram_tensor("oc", [B, S_SLICE, D], F32, kind="ExternalOutput").ap()

    EXP = mybir.ActivationFunctionType.Exp
    SCALE = DH ** -0.5

    with tile.TileContext(nc) as tc:
        with (
            tc.tile_pool(name="w", bufs=1) as wp,
            tc.tile_pool(name="qkt", bufs=1) as qktp,
            tc.tile_pool(name="vsb", bufs=1) as vsbp,
            tc.tile_pool(name="xtb", bufs=2) as xtbp,
            tc.tile_pool(name="pt", bufs=2) as ptp,
            tc.tile_pool(name="norm", bufs=2) as normp,
            tc.tile_pool(name="x2", bufs=1) as x2p,
            tc.tile_pool(name="outsb", bufs=2) as outp,
            tc.tile_pool(name="dram", bufs=1, space="DRAM") as dram,
            tc.tile_pool(name="dramsc", bufs=4, space="DRAM") as dramsc,
            tc.tile_pool(name="sps", bufs=2, space="PSUM") as sps,
            tc.tile_pool(name="avps", bufs=3, space="PSUM") as avps,
            tc.tile_pool(name="ops", bufs=1, space="PSUM") as ops,
        ):
            # ---- weights ----
            wqt_sb = wp.tile([128, N_CH * 128], BF16)
            nc.sync.dma_start(wqt_sb[:], wqt[:].rearrange("p c e -> p (c e)"))
            wkt_sb = wp.tile([128, N_CH * 128], BF16)
            nc.sync.dma_start(wkt_sb[:], wkt[:].rearrange("p c e -> p (c e)"))
            wvt_sb = wp.tile([128, N_CH * 128], BF16)
            nc.sync.dma_start(wvt_sb[:], wvt[:].rearrange("p c e -> p (c e)"))

            # ---- starting gun: tiny AllGather aligns the 8 cores ----
            gun_in = dram.tile([1, 16], F32, name="gun_in")
            gun_out = dram.tile([N_CORES, 16], F32, name="gun_out")
            gun_sb = wp.tile([1, 16], F32)
            nc.gpsimd.memset(gun_sb[:], 0.0)
            nc.sync.dma_start(gun_in[:], gun_sb[:])
            nc.gpsimd.collective_compute(
                "AllGather", mybir.AluOpType.bypass,
                replica_groups=[list(range(N_CORES))],
                ins=[gun_in[:]], outs=[gun_out[:]],
            )

            # ---- PE warmup while DMAs stream ----
            warm = wp.tile([128, 512], BF16)
            nc.gpsimd.memset(warm[:], 0.0)
            for i in range(48):
                wps = ops.tile([128, 512], F32, tag="o", name="wps")
                nc.tensor.matmul(wps[:], lhsT=warm[:, 0:128], rhs=warm[:],
                                 start=True, stop=True)

            xtb_sbs = [None, None]
            Qt, Kt, Vs = [], [], []
            for b in range(B):
                Qt.append(qktp.tile([128, S], BF16, tag=f"qt{b}", name=f"qt{b}"))
                Kt.append(qktp.tile([128, S], BF16, tag=f"kt{b}", name=f"kt{b}"))
                Vs.append(vsbp.tile([128, N_KC * 130], BF16, tag=f"v{b}",
                                    name=f"v{b}"))

            def emit_xtb_load(b):
                t_ = xtbp.tile([128, N_CH * S], BF16, tag="xtb", name="xtb_sb")
                xtb_sbs[b] = t_
                for ch in range(N_CH):
                    nc.sync.dma_start(t_[:, ch * S:(ch + 1) * S], xtb[b, :, ch, :])

            def emit_qk_fast(b):
                """ch-outer, 2 passes of 2 t-tiles; starts as chunks land."""
                for p_ in range(2):
                    q_ps = sps.tile([128, 1024], F32, tag="s", name="q_ps")
                    k_ps = sps.tile([128, 1024], F32, tag="s", name="k_ps")
                    for ch in range(N_CH):
                        for j in range(2):
                            t = 2 * p_ + j
                            rhs = xtb_sbs[b][:, ch * S + t * 512:
                                             ch * S + (t + 1) * 512]
                            nc.tensor.matmul(
                                q_ps[:, j * 512:(j + 1) * 512],
                                lhsT=wqt_sb[:, ch * 128:(ch + 1) * 128],
                                rhs=rhs, start=(ch == 0), stop=(ch == N_CH - 1),
                            )
                        for j in range(2):
                            t = 2 * p_ + j
                            rhs = xtb_sbs[b][:, ch * S + t * 512:
                                             ch * S + (t + 1) * 512]
                            nc.tensor.matmul(
                                k_ps[:, j * 512:(j + 1) * 512],
                                lhsT=wkt_sb[:, ch * 128:(ch + 1) * 128],
                                rhs=rhs, start=(ch == 0), stop=(ch == N_CH - 1),
                            )
                    nc.vector.tensor_copy(
                        Qt[b][:, p_ * 1024:(p_ + 1) * 1024], q_ps[:])
                    nc.vector.tensor_copy(
                        Kt[b][:, p_ * 1024:(p_ + 1) * 1024], k_ps[:])

            def emit_qk_slice(b, t):
                """ch-inner, one psum at a time (attention running)."""
                for which, w_sb, dst in (("q", wqt_sb, Qt[b]), ("k", wkt_sb, Kt[b])):
                    ps_ = sps.tile([128, 1024], F32, tag="s", name=f"{which}_ps1")
                    for ch in range(N_CH):
                        nc.tensor.matmul(
                            ps_[:, 0:512],
                            lhsT=w_sb[:, ch * 128:(ch + 1) * 128],
                            rhs=xtb_sbs[b][:, ch * S + t * 512:
                                           ch * S + (t + 1) * 512],
                            start=(ch == 0), stop=(ch == N_CH - 1),
                        )
                    nc.vector.tensor_copy(
                        dst[:, t * 512:(t + 1) * 512], ps_[:, 0:512])

            def emit_v(b, sts):
                v_sb = Vs[b]
                for st in sts:
                    v_ps = ops.tile([128, 512], F32, tag="o", name="v_ps")
                    for ch in range(N_CH):
                        nc.tensor.matmul(
                            v_ps[:, 0:128],
                            lhsT=xtb_sbs[b][:, ch * S + st * 128:
                                            ch * S + (st + 1) * 128],
                            rhs=wvt_sb[:, ch * 128:(ch + 1) * 128],
                            start=(ch == 0), stop=(ch == N_CH - 1),
                        )
                    dst = v_sb[:].rearrange("p (c o) -> p c o", o=65)[
                        :, 2 * st:2 * st + 2, 0:64
                    ]
                    nc.vector.tensor_copy(
                        dst, v_ps[:, 0:128].rearrange("p (h e) -> p h e", e=64)
                    )

            a2a_in = [dram.tile([N_CORES, 128, S_SLICE], BF16, tag=f"a2ai{b}",
                                name=f"a2ai{b}") for b in range(B)]
            a2a_out = [dram.tile([N_CORES, 128, S_SLICE], BF16, tag=f"a2ao{b}",
                                 name=f"a2ao{b}") for b in range(B)]

            def att_unit(b, h, t):
                hp = slice(h * 64, (h + 1) * 64)
                voff = h * 65
                av = avps.tile([65, 512], F32, tag="av", name="av")
                qs = slice(t * 512, (t + 1) * 512)
                for cc in range(N_KC // 2):
                    s_ps = sps.tile([128, 1024], F32, tag="s", name="s_ps")
                    for j in range(2):
                        c = 2 * cc + j
                        ks = slice(c * 128, (c + 1) * 128)
                        nc.tensor.matmul(
                            s_ps[:, j * 512:(j + 1) * 512],
                            lhsT=Kt[b][hp, ks], rhs=Qt[b][hp, qs],
                            start=True, stop=True,
                        )
                    p_sb = ptp.tile([128, 1024], BF16, tag="p", name="p_sb")
                    nc.scalar.activation(p_sb[:], s_ps[:], EXP, scale=SCALE)
                    for j in range(2):
                        c = 2 * cc + j
                        nc.tensor.matmul(
                            av[:],
                            lhsT=Vs[b][:, c * 130 + voff:c * 130 + voff + 65],
                            rhs=p_sb[:, j * 512:(j + 1) * 512],
                            start=(c == 0), stop=(c == N_KC - 1),
                            skip_group_check=True,
                        )
                # normalize: denom -> [64,8] reshape -> reciprocal -> broadcast
                den_sb = normp.tile([1, 512], F32, tag="dsb", name="den_sb")
                nc.vector.tensor_copy(den_sb[:], av[64:65, :])
                den_d = dramsc.tile([512], F32, tag="dend", name="den_d")
                nc.sync.dma_start(
                    den_d[:].rearrange("(a q) -> a q", a=1), den_sb[:])
                den64 = normp.tile([64, 8], F32, tag="d64", name="den64")
                nc.sync.dma_start(
                    den64[:], den_d[:].rearrange("(p q) -> p q", p=64))
                rec64 = normp.tile([64, 8], F32, tag="r64", name="rec64")
                nc.vector.reciprocal(rec64[:], den64[:])
                rsc = dramsc.tile([512], F32, tag="rsc", name="rsc")
                nc.sync.dma_start(
                    rsc[:].rearrange("(p q) -> p q", p=64), rec64[:])
                bcast = normp.tile([64, 512], F32, tag="bc", name="bcast")
                nc.sync.dma_start(
                    bcast[:],
                    rsc[:].rearrange("(a q) -> a q", a=1).broadcast_to([64, 512]),
                )
                o_sb = normp.tile([64, 512], BF16, tag="ob", name="o_sb")
                nc.vector.tensor_mul(o_sb[:], av[0:64, :], bcast[:])
                for j in range(2):
                    nc.sync.dma_start(
                        a2a_in[b][2 * t + j, hp, :],
                        o_sb[:, j * S_SLICE:(j + 1) * S_SLICE],
                    )

            def emit_a2a(b):
                nc.gpsimd.collective_compute(
                    "AllToAll", mybir.AluOpType.bypass,
                    replica_groups=[list(range(N_CORES))],
                    ins=[a2a_in[b][:]], outs=[a2a_out[b][:]],
                )

            x2_tiles = {}

            def emit_x2_loads(b):
                x2 = []
                for i in range(N_CH):
                    x2_sb = x2p.tile([128, S_SLICE], BF16, tag=f"x2_{b}_{i}",
                                     name=f"x2_{b}_{i}")
                    nc.sync.dma_start(x2_sb[:], a2a_out[b][i])
                    x2.append(x2_sb)
                x2_tiles[b] = x2

            def emit_outproj_piece(b, st, et, wot_sb, bb_sb):
                o_ps = ops.tile([128, 512], F32, tag="o", name="o_ps")
                for ch in range(N_CH):
                    nc.tensor.matmul(
                        o_ps[:],
                        lhsT=x2_tiles[b][ch][:, st * 128:(st + 1) * 128],
                        rhs=wot_sb[:, ch * D + et * 512:ch * D + (et + 1) * 512],
                        start=(ch == 0), stop=(ch == N_CH - 1),
                    )
                out_sb = outp.tile([128, 512], F32, tag="osb", name="out_sb")
                nc.vector.tensor_add(
                    out_sb[:], o_ps[:], bb_sb[:, et * 512:(et + 1) * 512])
                nc.sync.dma_start(
                    oc[b, st * 128:(st + 1) * 128, et * 512:(et + 1) * 512],
                    out_sb[:],
                )

            # ================= pipeline =================
            wot_sb = wp.tile([128, N_CH * D], BF16)
            nc.sync.dma_start(wot_sb[:], wot[:].rearrange("p c e -> p (c e)"))
            bb_sb = wp.tile([128, D], F32)
            nc.sync.dma_start(bb_sb[:], bb[:])

            emit_xtb_load(0)
            ones0 = Vs[0][:].rearrange("p (c o) -> p c o", o=65)[:, :, 64:65]
            nc.gpsimd.memset(ones0, 1.0)
            ones1 = Vs[1][:].rearrange("p (c o) -> p c o", o=65)[:, :, 64:65]
            nc.gpsimd.memset(ones1, 1.0)
            emit_qk_fast(0)
            emit_v(0, range(N_KC))
            emit_xtb_load(1)

            # batch-0 attention, batch-1 projection interleaved
            for t in range(N_QT):
                att_unit(0, 0, t)
                emit_qk_slice(1, t)
            for t in range(N_QT):
                att_unit(0, 1, t)
                emit_v(1, range(4 * t, 4 * t + 4))
            emit_a2a(0)
            emit_x2_loads(0)

            # batch-1 attention, batch-0 out-projection interleaved
            pieces = [(st, et) for st in range(S_SLICE // 128)
                      for et in range(D // 512)]
            for t in range(N_QT):
                att_unit(1, 0, t)
                st, et = pieces[t]
                emit_outproj_piece(0, st, et, wot_sb, bb_sb)
            for t in range(N_QT):
                att_unit(1, 1, t)
            emit_a2a(1)
            emit_x2_loads(1)
            for st, et in pieces:
                emit_outproj_piece(1, st, et, wot_sb, bb_sb)

    nc.compile()
    return nc


def _prep_chunked(a_t):
    """[Din, E] (already transposed) -> [128, Din//128, E] SBUF-chunk layout."""
    din, e = a_t.shape
    return np.ascontiguousarray(
        a_t.reshape(din // 128, 128, e).transpose(1, 0, 2)
    )


def kernel(x, w_qkv, w_out, b_out):
    global _compiled, last_results
    if _compiled is None:
        _compiled = _build()
    nc = _compiled

    x = np.asarray(x, dtype=np.float32)
    w_qkv = np.asarray(w_qkv, dtype=np.float32)
    w_out = np.asarray(w_out, dtype=np.float32)
    b_out = np.asarray(b_out, dtype=np.float32)

    # x^T in chunk layout: [B, 128, N_CH, S], bf16
    xt_full = x.transpose(0, 2, 1)  # [B, D, S]
    xtb_prep = np.ascontiguousarray(
        xt_full.reshape(B, N_CH, 128, S).transpose(0, 2, 1, 3)
    ).astype(ml_dtypes.bfloat16)

    wot_prep = _prep_chunked(np.ascontiguousarray(w_out.T)).astype(ml_dtypes.bfloat16)
    bb_np = np.ascontiguousarray(np.broadcast_to(b_out, (128, D)))

    in_maps = []
    for c in range(N_CORES):
        hA, hB = HEADS_PER_CORE * c, HEADS_PER_CORE * c + 1
        rows = np.r_[hA * DH:(hA + 1) * DH, hB * DH:(hB + 1) * DH]
        wq = w_qkv[rows, :]               # [128, D]
        wk = w_qkv[D + rows, :]
        wv = w_qkv[2 * D + rows, :]
        in_maps.append({
            "xtb": xtb_prep,
            "wqt": _prep_chunked(np.ascontiguousarray(wq.T)).astype(ml_dtypes.bfloat16),
            "wkt": _prep_chunked(np.ascontiguousarray(wk.T)).astype(ml_dtypes.bfloat16),
            "wvt": _prep_chunked(np.ascontiguousarray(wv.T)).astype(ml_dtypes.bfloat16),
            "wot": wot_prep,
            "bb": bb_np,
        })

    last_results = bass_utils.run_bass_kernel_spmd(
        nc, in_maps, core_ids=list(range(N_CORES))
    )
    out = np.concatenate(
        [last_results.results[c]["oc"] for c in range(N_CORES)], axis=1
    )
    return out
